# revision 41
# baseline (speedup 1.0000x reference)
"""NNCLR forward loss kernel for 8x TRN2 NeuronCores (~127us HW, 4.6x base).

Launch A (8 cores): fp8e4 DoubleRow sims (K=256 in one matmul pass, 0.5
cyc/row) of both projections (1024 rows) against the core's queue shard
(12288 rows, host-prescaled x8 into e4m3 range). Per tile of 128 rows x 6
blocks of 2048 cols: 3 blocks get a DVE grouped max from PSUM (groups of
32, fp32), the other 3 are converted to bf16 by Act and DMA'd out verbatim.
Host screens all reported maxima within an eps covering the fp8 sims error
(sigma ~0.051), rescores candidate positions exactly in fp64, and picks the
global top-1 per row (first-occurrence ties). fp8 only screens candidates -
the final argmax decision is always fp64-exact on the host.

Launch C (8 cores): each core computes 2 of the 16 [128, 512] logit row-tiles
(fp32r matmuls on host-prepped transposed/normalized/temperature-folded
operands), log-softmax + diagonal extraction, returning a [128, 2] loss
slice; host concatenates the 16 tiles into the [2048] loss.
"""

import numpy as np

import concourse.bass as bass
import concourse.mybir as mybir
from concourse.tile import TileContext

import bass_rust as _br
import concourse.tile as _tile_mod


def _patched_drain_and_barrier(self, tick_clock, wait_clock):
    """Walrus here only allows 2 sem waits per instruction; split the
    Tile tail drain's wait list across extra drain instructions."""
    drain_inst = self.nc.sync.drain()
    wait_clock.add_sem_waits(
        drain_inst.ins, _br.ScopedClock({None: tick_clock.global_clock})
    )
    si = drain_inst.ins.sync_info
    if si is not None and si.on_wait and len(si.on_wait) > 1:
        waits = list(si.on_wait)
        drain_inst.ins.sync_info = _br.SyncInfo(on_wait=waits[:1], on_update=list(si.on_update))
        for i in range(1, len(waits)):
            extra = self.nc.sync.drain()
            extra.ins.sync_info = _br.SyncInfo(on_wait=waits[i : i + 1], on_update=[])
    self.nc.all_engine_barrier()
    assert self.sems is not None
    popped = self.nc._tile_sem_poison_stack.pop()
    assert popped is self._sem_poison
    self.nc.clear_and_free_semaphores(list(self.sems.allocated().values()))
    self.nc.all_engine_barrier()


_tile_mod.TileContext._drain_and_barrier = _patched_drain_and_barrier


def _split_multi_waits(nc):
    """This walrus build allows only one sync-wait per instruction; hoist
    extra waits onto NOPs inserted just before, on the same engine."""
    n_split = 0
    for f in nc.m.functions:
        for bb in f.blocks:
            il = bb.instructions
            i = 0
            while i < len(il):
                inst = il[i]
                si = inst.sync_info
                if si is not None and si.on_wait and len(si.on_wait) > 1:
                    waits = list(si.on_wait)
                    nops = []
                    for w in waits[:-1]:
                        nop = mybir.InstNoOp(
                            name=f"waitsplit-{nc.next_id()}",
                            engine=inst.engine,
                            ins=[],
                            outs=[],
                            sync_info=_br.SyncInfo(on_wait=[w], on_update=[]),
                        )
                        nc.register_instruction(nop, overwrite=True)
                        nops.append(nop)
                    inst.sync_info = _br.SyncInfo(
                        on_wait=[waits[-1]], on_update=list(si.on_update)
                    )
                    il[i:i] = nops
                    i += len(nops)
                    n_split += 1
                i += 1
    return n_split


F32 = mybir.dt.float32
F32R = mybir.dt.float32r
BF16 = mybir.dt.bfloat16
AF = mybir.ActivationFunctionType
AX = mybir.AxisListType

B = 512
D = 256
B2 = 2 * B  # 1024 combined rows (p1 then p2)
NCORES = 8
Q_FULL = 98304
QS = Q_FULL // NCORES  # 12288
NT = B2 // 128  # 8 row tiles
NB = QS // 2048  # 6 col blocks of 2048
# Per tile t: 3 blocks get a DVE grouped-max (fp32 from PSUM, groups of 32);
# the other 3 are converted to bf16 by Act and DMA'd out verbatim (exact
# positions for the host). The stream order is (b outer, t inner), so block
# (b, t) lands in PSUM slot t%2; keying the type on t//2 parity makes each
# slot alternate DVE/Act consumers within every round (slot period
# DVE+mm / Act+mm interleaved instead of whole rounds of the slower DVE).
# Tile 7 is flipped so the final block (b=5, t=7) ends on the cheaper Act
# path instead of a trailing DVE reduce.
DIRECT_BS = [
    [b for b in range(NB) if (b + (t // 2) + (1 if t == 7 else 0)) % 2 == 0]
    for t in range(NT)
]
ACT_BS = [[b for b in range(NB) if b not in DIRECT_BS[t]] for t in range(NT)]
GROUP = 32
# fp32r abs error bound: measured 7.5e-4 max (||p||~16 rows); 3x margin
DELTA_R = 2.5e-3
# fp8 (e4m3) sims: per-product RMS err ~5.1% -> sims err sigma ~0.051 abs;
# screening eps uses ~4.3 sigma per side
DELTA_8 = 0.26
FP8 = True  # use fp8e4 DoubleRow matmuls in launch A
Q8_SCALE = 8.0  # pre-scale on q before fp8 cast (host); undone on gm/ab


def build_nc_A():
    in_dt = mybir.dt.float8e4 if FP8 else F32R
    nc = bass.Bass(num_devices=NCORES, debug=False)
    pT = nc.declare_dram_parameter("pT", [D, B2], in_dt, isOutput=False)
    qT = nc.declare_dram_parameter("qT", [D, QS], in_dt, isOutput=False)
    gmax_out = nc.declare_dram_parameter(
        "gmax", [128, NT, 3, 2048 // GROUP], F32, isOutput=True
    )
    accb_out = nc.declare_dram_parameter("accb", [128, NT, 3, 2048], BF16, isOutput=True)

    with TileContext(nc) as tc:
        with (
            tc.tile_pool(name="persist", bufs=1) as pp,
            tc.tile_pool(name="qstream", bufs=4) as qp,
            tc.tile_pool(name="bconv", bufs=4) as bp,
            tc.tile_pool(name="psumA", bufs=2, space="PSUM") as psA,
        ):
            pT_sb = pp.tile([128, 2, B2], in_dt)
            pT3 = pT.ap().rearrange("(k p) b -> p k b", p=128)
            # tiny first pieces on the scalar queue unblock matmul 0
            nc.scalar.dma_start(pT_sb[:, 0, 0:512], pT3[:, 0, 0:512])
            nc.scalar.dma_start(pT_sb[:, 1, 0:128], pT3[:, 1, 0:128])
            nc.scalar.dma_start(pT_sb[:, 0, 512:B2], pT3[:, 0, 512:B2])
            nc.scalar.dma_start(pT_sb[:, 1, 128:B2], pT3[:, 1, 128:B2])
            qT3 = qT.ap().rearrange("(k p) q -> p k q", p=128)
            gmax_sb = pp.tile([128, NT, 3, 2048 // GROUP], F32)

            q_tiles = []
            for b in range(NB):
                qt = qp.tile([128, 2, 2048], in_dt, tag="q", name=f"q_{b}")
                if b == 0:
                    # first block in 4 chunk-DMAs so matmul 0 starts ASAP
                    for c4 in range(4):
                        nc.sync.dma_start(
                            qt[:, :, c4 * 512 : (c4 + 1) * 512],
                            qT3[:, :, c4 * 512 : (c4 + 1) * 512],
                        )
                else:
                    nc.sync.dma_start(qt[:], qT3[:, :, b * 2048 : (b + 1) * 2048])
                q_tiles.append(qt)

            for b in range(NB):
                qt = q_tiles[b]
                for t in range(NT):
                    ps = psA.tile([128, 2048], F32, tag="blk", name=f"ps_{t}_{b}")
                    if FP8:
                        # DoubleRow: both K halves contracted in one pass
                        for c4 in range(4):
                            nc.tensor.matmul(
                                ps[:, c4 * 512 : (c4 + 1) * 512],
                                pT_sb[:, :, t * 128 : (t + 1) * 128],
                                qt[:, :, c4 * 512 : (c4 + 1) * 512],
                                start=True,
                                stop=True,
                                perf_mode=mybir.MatmulPerfMode.DoubleRow,
                            )
                    else:
                        for k in range(2):
                            for c4 in range(4):
                                nc.tensor.matmul(
                                    ps[:, c4 * 512 : (c4 + 1) * 512],
                                    pT_sb[:, k, t * 128 : (t + 1) * 128],
                                    qt[:, k, c4 * 512 : (c4 + 1) * 512],
                                    start=(k == 0),
                                    stop=(k == 1),
                                )
                    if b in DIRECT_BS[t]:
                        slot = DIRECT_BS[t].index(b)
                        nc.vector.reduce_max(
                            gmax_sb[:, t, slot, :],
                            ps.rearrange("p (g k) -> p g k", k=GROUP),
                            axis=AX.X,
                        )
                        if b == DIRECT_BS[t][-1]:
                            nc.sync.dma_start(
                                gmax_out.ap()[:, t, :, :], gmax_sb[:, t, :, :]
                            )
                    else:
                        slot = ACT_BS[t].index(b)
                        bc = bp.tile([128, 2048], BF16, tag="bc", name=f"bc_{t}_{b}")
                        nc.scalar.copy(bc[:], ps[:])
                        nc.sync.dma_start(accb_out.ap()[:, t, slot, :], bc[:])

    _split_multi_waits(nc)
    return nc


def build_nc_C():
    nc = bass.Bass(num_devices=NCORES, debug=False)
    lhsT = nc.declare_dram_parameter("lhsT", [D, 256], F32R, isOutput=False)
    rhsT = nc.declare_dram_parameter("rhsT", [D, B], F32R, isOutput=False)
    lhsN = nc.declare_dram_parameter("lhsN", [128, 2, D], F32, isOutput=False)
    rhsN = nc.declare_dram_parameter("rhsN", [128, 2, D], F32, isOutput=False)
    loss_out = nc.declare_dram_parameter("loss", [128, 2], F32, isOutput=True)

    with TileContext(nc) as tc:
        with (
            tc.tile_pool(name="persist", bufs=1) as pp,
            tc.tile_pool(name="scr", bufs=2) as sp,
            tc.tile_pool(name="psumC", bufs=2, space="PSUM") as psC,
        ):
            rhsT3 = rhsT.ap().rearrange("(k p) c -> p k c", p=128)
            lhsT3 = lhsT.ap().rearrange("(k p) c -> p k c", p=128)
            lhsT_sb = pp.tile([128, 2, 256], F32R)
            rhsT_sb = pp.tile([128, 2, B], F32R)
            # matmul-0 operands first on both HWDGE queues: rhsT k0 on sync,
            # the j=0 stationary slice of lhsT k0 on scalar
            nc.sync.dma_start(rhsT_sb[:, 0, :], rhsT3[:, 0, :])
            nc.scalar.dma_start(lhsT_sb[:, 0, 0:128], lhsT3[:, 0, 0:128])
            nc.scalar.dma_start(lhsT_sb[:, 0, 128:256], lhsT3[:, 0, 128:256])
            nc.scalar.dma_start(lhsT_sb[:, 1, :], lhsT3[:, 1, :])
            nc.sync.dma_start(rhsT_sb[:, 1, :], rhsT3[:, 1, :])
            lhsN_sb = pp.tile([128, 2, D], F32)
            nc.scalar.dma_start(lhsN_sb[:], lhsN.ap())
            rhsN_sb = pp.tile([128, 2, D], F32)
            nc.sync.dma_start(rhsN_sb[:], rhsN.ap())

            M = pp.tile([128, 2], F32)
            negM = pp.tile([128, 2], F32)
            S = pp.tile([128, 2], F32)
            dmul = pp.tile([128, 2, D], F32)
            dg = pp.tile([128, 2], F32)
            for j in range(2):
                psc = psC.tile([128, B], F32, tag="psc", name=f"psc_{j}")
                for k in range(2):
                    nc.tensor.matmul(
                        psc[:],
                        lhsT_sb[:, k, j * 128 : (j + 1) * 128],
                        rhsT_sb[:, k, :],
                        start=(k == 0),
                        stop=(k == 1),
                    )
                nc.vector.reduce_max(M[:, j : j + 1], psc[:], axis=AX.X)
                nc.vector.tensor_scalar_mul(negM[:, j : j + 1], M[:, j : j + 1], -1.0)
                escr = sp.tile([128, B], F32, tag="escr", name=f"escr_{j}")
                nc.scalar.activation(
                    escr[:], psc[:], AF.Exp,
                    bias=negM[:, j : j + 1], scale=1.0,
                    accum_out=S[:, j : j + 1],
                )
                # diag dot products overlap with the matmul/softmax chain
                nc.vector.tensor_mul(dmul[:, j, :], lhsN_sb[:, j, :], rhsN_sb[:, j, :])
                nc.vector.reduce_sum(dg[:, j : j + 1], dmul[:, j, :], axis=AX.X)
            lnS = pp.tile([128, 2], F32)
            nc.scalar.activation(lnS[:], S[:], AF.Ln)
            lossT = pp.tile([128, 2], F32)
            nc.vector.tensor_add(lossT[:], lnS[:], M[:])
            nc.vector.tensor_sub(lossT[:], lossT[:], dg[:])
            nc.sync.dma_start(loss_out.ap(), lossT[:])

    _split_multi_waits(nc)
    return nc


_CACHE = {}


def _get_nc(which):
    if which not in _CACHE:
        _CACHE[which] = build_nc_A() if which == "A" else build_nc_C()
    return _CACHE[which]


def _prep_A_inmaps(p1, p2, fq):
    pT_full = np.ascontiguousarray(np.concatenate([p1, p2], axis=0).T)
    if FP8:
        import ml_dtypes

        pT_full = pT_full.astype(ml_dtypes.float8_e4m3)
        return [
            {
                "pT": pT_full,
                "qT": np.ascontiguousarray(
                    (fq[c * QS : (c + 1) * QS].T * np.float32(Q8_SCALE))
                ).astype(ml_dtypes.float8_e4m3),
            }
            for c in range(NCORES)
        ]
    return [
        {"pT": pT_full, "qT": np.ascontiguousarray(fq[c * QS : (c + 1) * QS].T)}
        for c in range(NCORES)
    ]


def _host_top1(resA_results, p1, p2, fq):
    """Screen device maxima, rescore candidates in fp64, return jglob[1024]."""
    gm = np.stack([np.asarray(resA_results[c]["gmax"]) for c in range(NCORES)])
    ab = np.stack(
        [np.asarray(resA_results[c]["accb"]).astype(np.float32) for c in range(NCORES)]
    )
    # [c, 128p, t, ...] -> row r = t*128 + p
    gm = gm.transpose(0, 2, 1, 3, 4).reshape(NCORES, B2, 3, 2048 // GROUP)
    ab = ab.transpose(0, 2, 1, 3, 4).reshape(NCORES, B2, 3, 2048)
    if FP8:
        gm /= np.float32(Q8_SCALE)
        ab /= np.float32(Q8_SCALE)

    delta = DELTA_8 if FP8 else DELTA_R
    Mt = np.maximum(gm.max(axis=(0, 2, 3)), ab.max(axis=(0, 2, 3)))  # [B2]
    eps = 2 * delta + np.abs(Mt) * 2.0 ** -8 + 1e-4
    thresh = Mt - eps

    dir_b = np.asarray(DIRECT_BS, dtype=np.int64)  # [NT, 3]
    act_b = np.asarray(ACT_BS, dtype=np.int64)  # [NT, 3]
    rows_list, pos_list = [], []
    c_i, r_i, s_i, g_i = np.nonzero(gm >= thresh[None, :, None, None])
    if len(c_i):
        base = (
            c_i.astype(np.int64) * QS
            + dir_b[r_i // 128, s_i] * 2048
            + g_i.astype(np.int64) * GROUP
        )
        pos = (base[:, None] + np.arange(GROUP, dtype=np.int64)[None, :]).reshape(-1)
        rows = np.repeat(r_i.astype(np.int64), GROUP)
        rows_list.append(rows)
        pos_list.append(pos)
    c_i, r_i, s_i, p_i = np.nonzero(ab >= thresh[None, :, None, None])
    if len(c_i):
        pos = (
            c_i.astype(np.int64) * QS
            + act_b[r_i // 128, s_i] * 2048
            + p_i.astype(np.int64)
        )
        rows = r_i.astype(np.int64)
        rows_list.append(rows)
        pos_list.append(pos)
    rows = np.concatenate(rows_list)
    pos = np.concatenate(pos_list)

    P64 = np.concatenate([p1, p2], axis=0).astype(np.float64)
    s = np.einsum("kd,kd->k", fq[pos].astype(np.float64), P64[rows])

    # first-occurrence argmax per row: sort by (row, pos), take first pos
    # attaining the row max
    order = np.lexsort((pos, rows))
    rows_s, pos_s, s_s = rows[order], pos[order], s[order]
    jglob = np.empty(B2, dtype=np.int64)
    starts = np.searchsorted(rows_s, np.arange(B2), side="left")
    ends = np.searchsorted(rows_s, np.arange(B2), side="right")
    for r in range(B2):
        sl = slice(starts[r], ends[r])
        sv = s_s[sl]
        jglob[r] = pos_s[sl][np.argmax(sv)]
    return jglob


def _prep_C_inmaps(p1, p2, nn, temp):
    def l2n(x):
        n = np.sqrt((x.astype(np.float64) ** 2).sum(axis=1, keepdims=True))
        return (x / np.maximum(n, 1e-12)).astype(np.float32)

    p1n = l2n(p1)
    p2n = l2n(p2)
    inv_t = np.float32(1.0) / np.float32(temp)
    p1s = (p1n * inv_t).astype(np.float32)
    p2s = (p2n * inv_t).astype(np.float32)
    nn1, nn2 = nn[:B], nn[B:]
    nn1_adj = ((nn1 - p1n) + p1n).astype(np.float32)
    nn2_adj = ((nn2 - p2n) + p2n).astype(np.float32)

    mats = [(nn1_adj, p2s), (p2s, nn1_adj), (nn2_adj, p1s), (p1s, nn2_adj)]
    in_maps = []
    for c in range(NCORES):
        m = c // 2
        i0 = (c % 2) * 2
        lhs, rhs = mats[m]
        lhsT = np.ascontiguousarray(lhs.T[:, i0 * 128 : (i0 + 2) * 128])
        rhsT = np.ascontiguousarray(rhs.T)
        lhsN = np.ascontiguousarray(
            lhs.reshape(4, 128, D)[i0 : i0 + 2].transpose(1, 0, 2)
        )
        rhsN = np.ascontiguousarray(
            rhs.reshape(4, 128, D)[i0 : i0 + 2].transpose(1, 0, 2)
        )
        in_maps.append({"lhsT": lhsT, "rhsT": rhsT, "lhsN": lhsN, "rhsN": rhsN})
    return in_maps


def kernel(projections_1, projections_2, feature_queue, temperature):
    from concourse.bass_utils import run_bass_kernel_spmd

    p1 = np.ascontiguousarray(projections_1, dtype=np.float32)
    p2 = np.ascontiguousarray(projections_2, dtype=np.float32)
    fq = np.ascontiguousarray(feature_queue, dtype=np.float32)

    ncA = _get_nc("A")
    resA = run_bass_kernel_spmd(ncA, _prep_A_inmaps(p1, p2, fq), core_ids=list(range(NCORES)))
    jglob = _host_top1(resA.results, p1, p2, fq)
    nn = fq[jglob]

    ncC = _get_nc("C")
    resC = run_bass_kernel_spmd(
        ncC, _prep_C_inmaps(p1, p2, nn, temperature), core_ids=list(range(NCORES))
    )
    loss = np.empty(4 * B, dtype=np.float32)
    for c in range(NCORES):
        out = np.asarray(resC.results[c]["loss"], dtype=np.float32)  # [128, 2]
        for j in range(2):
            rt = 2 * c + j
            loss[rt * 128 : (rt + 1) * 128] = out[:, j]
    return loss


# revision 45
# speedup vs baseline: 1.0202x; 1.0202x over previous
"""NNCLR forward loss kernel for 8x TRN2 NeuronCores (~127us HW, 4.6x base).

Launch A (8 cores): fp8e4 DoubleRow sims (K=256 in one matmul pass, 0.5
cyc/row) of both projections (1024 rows) against the core's queue shard
(12288 rows, host-prescaled x8 into e4m3 range). Per tile of 128 rows x 6
blocks of 2048 cols: 3 blocks get a DVE grouped max from PSUM (groups of
32, fp32), the other 3 are converted to bf16 by Act and DMA'd out verbatim.
Host screens all reported maxima within an eps covering the fp8 sims error
(sigma ~0.051), rescores candidate positions exactly in fp64, and picks the
global top-1 per row (first-occurrence ties). fp8 only screens candidates -
the final argmax decision is always fp64-exact on the host.

Launch C (8 cores): each core computes 2 of the 16 [128, 512] logit row-tiles
(fp32r matmuls on host-prepped transposed/normalized/temperature-folded
operands), log-softmax + diagonal extraction, returning a [128, 2] loss
slice; host concatenates the 16 tiles into the [2048] loss.
"""

import numpy as np

import concourse.bass as bass
import concourse.mybir as mybir
from concourse.tile import TileContext

import bass_rust as _br
import concourse.tile as _tile_mod


def _patched_drain_and_barrier(self, tick_clock, wait_clock):
    """Walrus here only allows 2 sem waits per instruction; split the
    Tile tail drain's wait list across extra drain instructions."""
    drain_inst = self.nc.sync.drain()
    wait_clock.add_sem_waits(
        drain_inst.ins, _br.ScopedClock({None: tick_clock.global_clock})
    )
    si = drain_inst.ins.sync_info
    if si is not None and si.on_wait and len(si.on_wait) > 1:
        waits = list(si.on_wait)
        drain_inst.ins.sync_info = _br.SyncInfo(on_wait=waits[:1], on_update=list(si.on_update))
        for i in range(1, len(waits)):
            extra = self.nc.sync.drain()
            extra.ins.sync_info = _br.SyncInfo(on_wait=waits[i : i + 1], on_update=[])
    self.nc.all_engine_barrier()
    assert self.sems is not None
    popped = self.nc._tile_sem_poison_stack.pop()
    assert popped is self._sem_poison
    self.nc.clear_and_free_semaphores(list(self.sems.allocated().values()))
    self.nc.all_engine_barrier()


_tile_mod.TileContext._drain_and_barrier = _patched_drain_and_barrier


def _split_multi_waits(nc):
    """This walrus build allows only one sync-wait per instruction; hoist
    extra waits onto NOPs inserted just before, on the same engine."""
    n_split = 0
    for f in nc.m.functions:
        for bb in f.blocks:
            il = bb.instructions
            i = 0
            while i < len(il):
                inst = il[i]
                si = inst.sync_info
                if si is not None and si.on_wait and len(si.on_wait) > 1:
                    waits = list(si.on_wait)
                    nops = []
                    for w in waits[:-1]:
                        nop = mybir.InstNoOp(
                            name=f"waitsplit-{nc.next_id()}",
                            engine=inst.engine,
                            ins=[],
                            outs=[],
                            sync_info=_br.SyncInfo(on_wait=[w], on_update=[]),
                        )
                        nc.register_instruction(nop, overwrite=True)
                        nops.append(nop)
                    inst.sync_info = _br.SyncInfo(
                        on_wait=[waits[-1]], on_update=list(si.on_update)
                    )
                    il[i:i] = nops
                    i += len(nops)
                    n_split += 1
                i += 1
    return n_split


F32 = mybir.dt.float32
F32R = mybir.dt.float32r
BF16 = mybir.dt.bfloat16
AF = mybir.ActivationFunctionType
AX = mybir.AxisListType

B = 512
D = 256
B2 = 2 * B  # 1024 combined rows (p1 then p2)
NCORES = 8
Q_FULL = 98304
QS = Q_FULL // NCORES  # 12288
NT = B2 // 128  # 8 row tiles
NB = QS // 2048  # 6 col blocks of 2048
# Per tile t: 3 blocks get a DVE grouped-max (fp32 from PSUM, groups of 32);
# the other 3 are converted to bf16 by Act and DMA'd out verbatim (exact
# positions for the host). The stream order is (b outer, t inner), so block
# (b, t) lands in PSUM slot t%2; keying the type on t//2 parity makes each
# slot alternate DVE/Act consumers within every round (slot period
# DVE+mm / Act+mm interleaved instead of whole rounds of the slower DVE).
# Tile 7 is flipped so the final block (b=5, t=7) ends on the cheaper Act
# path instead of a trailing DVE reduce.
DIRECT_BS = [
    [b for b in range(NB) if (b + (t // 2) + (1 if t == 7 else 0)) % 2 == 0]
    for t in range(NT)
]
ACT_BS = [[b for b in range(NB) if b not in DIRECT_BS[t]] for t in range(NT)]
GROUP = 32
# fp32r abs error bound: measured 7.5e-4 max (||p||~16 rows); 3x margin
DELTA_R = 2.5e-3
# fp8 (e4m3) sims: per-product RMS err ~5.1% -> sims err sigma ~0.051 abs;
# screening eps uses ~4.3 sigma per side
DELTA_8 = 0.26
FP8 = True  # use fp8e4 DoubleRow matmuls in launch A
Q8_SCALE = 8.0  # pre-scale on q before fp8 cast (host); undone on gm/ab


def build_nc_A():
    in_dt = mybir.dt.float8e4 if FP8 else F32R
    nc = bass.Bass(num_devices=NCORES, debug=False)
    pT = nc.declare_dram_parameter("pT", [D, B2], in_dt, isOutput=False)
    qT = nc.declare_dram_parameter("qT", [D, QS], in_dt, isOutput=False)
    gmax_out = nc.declare_dram_parameter(
        "gmax", [128, NT, 3, 2048 // GROUP], F32, isOutput=True
    )
    accb_out = nc.declare_dram_parameter("accb", [128, NT, 3, 2048], BF16, isOutput=True)

    with TileContext(nc) as tc:
        with (
            tc.tile_pool(name="persist", bufs=1) as pp,
            tc.tile_pool(name="qstream", bufs=3) as qp,
            tc.tile_pool(name="bconv", bufs=3) as bp,
            tc.tile_pool(name="psumA", bufs=2, space="PSUM") as psA,
        ):
            pT_sb = pp.tile([128, 2, B2], in_dt)
            pT3 = pT.ap().rearrange("(k p) b -> p k b", p=128)
            # tiny first pieces on the scalar queue unblock matmul 0
            nc.scalar.dma_start(pT_sb[:, 0, 0:512], pT3[:, 0, 0:512])
            nc.scalar.dma_start(pT_sb[:, 1, 0:128], pT3[:, 1, 0:128])
            nc.scalar.dma_start(pT_sb[:, 0, 512:B2], pT3[:, 0, 512:B2])
            nc.scalar.dma_start(pT_sb[:, 1, 128:B2], pT3[:, 1, 128:B2])
            qT3 = qT.ap().rearrange("(k p) q -> p k q", p=128)
            gmax_sb = pp.tile([128, NT, 3, 2048 // GROUP], F32)

            q_tiles = []
            for b in range(NB):
                qt = qp.tile([128, 2, 2048], in_dt, tag="q", name=f"q_{b}")
                if b == 0:
                    # first block in 4 chunk-DMAs so matmul 0 starts ASAP
                    for c4 in range(4):
                        nc.sync.dma_start(
                            qt[:, :, c4 * 512 : (c4 + 1) * 512],
                            qT3[:, :, c4 * 512 : (c4 + 1) * 512],
                        )
                else:
                    nc.sync.dma_start(qt[:], qT3[:, :, b * 2048 : (b + 1) * 2048])
                q_tiles.append(qt)

            for b in range(NB):
                qt = q_tiles[b]
                for t in range(NT):
                    ps = psA.tile([128, 2048], F32, tag="blk", name=f"ps_{t}_{b}")
                    if FP8:
                        # DoubleRow: both K halves contracted in one pass
                        for c4 in range(4):
                            nc.tensor.matmul(
                                ps[:, c4 * 512 : (c4 + 1) * 512],
                                pT_sb[:, :, t * 128 : (t + 1) * 128],
                                qt[:, :, c4 * 512 : (c4 + 1) * 512],
                                start=True,
                                stop=True,
                                perf_mode=mybir.MatmulPerfMode.DoubleRow,
                            )
                    else:
                        for k in range(2):
                            for c4 in range(4):
                                nc.tensor.matmul(
                                    ps[:, c4 * 512 : (c4 + 1) * 512],
                                    pT_sb[:, k, t * 128 : (t + 1) * 128],
                                    qt[:, k, c4 * 512 : (c4 + 1) * 512],
                                    start=(k == 0),
                                    stop=(k == 1),
                                )
                    if b in DIRECT_BS[t]:
                        slot = DIRECT_BS[t].index(b)
                        nc.vector.reduce_max(
                            gmax_sb[:, t, slot, :],
                            ps.rearrange("p (g k) -> p g k", k=GROUP),
                            axis=AX.X,
                        )
                        if b == DIRECT_BS[t][-1]:
                            nc.sync.dma_start(
                                gmax_out.ap()[:, t, :, :], gmax_sb[:, t, :, :]
                            )
                    else:
                        slot = ACT_BS[t].index(b)
                        bc = bp.tile([128, 2048], BF16, tag="bc", name=f"bc_{t}_{b}")
                        nc.scalar.copy(bc[:], ps[:])
                        nc.sync.dma_start(accb_out.ap()[:, t, slot, :], bc[:])

    _split_multi_waits(nc)
    return nc


def build_nc_C():
    nc = bass.Bass(num_devices=NCORES, debug=False)
    lhsT = nc.declare_dram_parameter("lhsT", [D, 256], F32R, isOutput=False)
    rhsT = nc.declare_dram_parameter("rhsT", [D, B], F32R, isOutput=False)
    lhsN = nc.declare_dram_parameter("lhsN", [128, 2, D], F32, isOutput=False)
    rhsN = nc.declare_dram_parameter("rhsN", [128, 2, D], F32, isOutput=False)
    loss_out = nc.declare_dram_parameter("loss", [128, 2], F32, isOutput=True)

    with TileContext(nc) as tc:
        with (
            tc.tile_pool(name="persist", bufs=1) as pp,
            tc.tile_pool(name="scr", bufs=2) as sp,
            tc.tile_pool(name="psumC", bufs=2, space="PSUM") as psC,
        ):
            rhsT3 = rhsT.ap().rearrange("(k p) c -> p k c", p=128)
            lhsT3 = lhsT.ap().rearrange("(k p) c -> p k c", p=128)
            lhsT_sb = pp.tile([128, 2, 256], F32R)
            rhsT_sb = pp.tile([128, 2, B], F32R)
            # matmul-0 operands first on both HWDGE queues: rhsT k0 on sync,
            # the j=0 k=0 stationary slice of lhsT on scalar
            nc.sync.dma_start(rhsT_sb[:, 0, :], rhsT3[:, 0, :])
            nc.scalar.dma_start(lhsT_sb[:, 0, 0:128], lhsT3[:, 0, 0:128])
            nc.scalar.dma_start(lhsT_sb[:, 0, 128:256], lhsT3[:, 0, 128:256])
            nc.scalar.dma_start(lhsT_sb[:, 1, :], lhsT3[:, 1, :])
            nc.sync.dma_start(rhsT_sb[:, 1, :], rhsT3[:, 1, :])
            lhsN_sb = pp.tile([128, 2, D], F32)
            nc.scalar.dma_start(lhsN_sb[:], lhsN.ap())
            rhsN_sb = pp.tile([128, 2, D], F32)
            nc.sync.dma_start(rhsN_sb[:], rhsN.ap())

            M = pp.tile([128, 2], F32)
            negM = pp.tile([128, 2], F32)
            S = pp.tile([128, 2], F32)
            dmul = pp.tile([128, 2, D], F32)
            dg = pp.tile([128, 2], F32)
            for j in range(2):
                psc = psC.tile([128, B], F32, tag="psc", name=f"psc_{j}")
                for k in range(2):
                    nc.tensor.matmul(
                        psc[:],
                        lhsT_sb[:, k, j * 128 : (j + 1) * 128],
                        rhsT_sb[:, k, :],
                        start=(k == 0),
                        stop=(k == 1),
                    )
                nc.vector.reduce_max(M[:, j : j + 1], psc[:], axis=AX.X)
                nc.vector.tensor_scalar_mul(negM[:, j : j + 1], M[:, j : j + 1], -1.0)
                escr = sp.tile([128, B], F32, tag="escr", name=f"escr_{j}")
                nc.scalar.activation(
                    escr[:], psc[:], AF.Exp,
                    bias=negM[:, j : j + 1], scale=1.0,
                    accum_out=S[:, j : j + 1],
                )
                # diag dot products overlap with the matmul/softmax chain
                nc.vector.tensor_mul(dmul[:, j, :], lhsN_sb[:, j, :], rhsN_sb[:, j, :])
                nc.vector.reduce_sum(dg[:, j : j + 1], dmul[:, j, :], axis=AX.X)
            lnS = pp.tile([128, 2], F32)
            nc.scalar.activation(lnS[:], S[:], AF.Ln)
            lossT = pp.tile([128, 2], F32)
            nc.vector.tensor_add(lossT[:], lnS[:], M[:])
            nc.vector.tensor_sub(lossT[:], lossT[:], dg[:])
            nc.sync.dma_start(loss_out.ap(), lossT[:])

    _split_multi_waits(nc)
    return nc


_CACHE = {}


def _get_nc(which):
    if which not in _CACHE:
        _CACHE[which] = build_nc_A() if which == "A" else build_nc_C()
    return _CACHE[which]


def _prep_A_inmaps(p1, p2, fq):
    pT_full = np.ascontiguousarray(np.concatenate([p1, p2], axis=0).T)
    if FP8:
        import ml_dtypes

        pT_full = pT_full.astype(ml_dtypes.float8_e4m3)
        return [
            {
                "pT": pT_full,
                "qT": np.ascontiguousarray(
                    (fq[c * QS : (c + 1) * QS].T * np.float32(Q8_SCALE))
                ).astype(ml_dtypes.float8_e4m3),
            }
            for c in range(NCORES)
        ]
    return [
        {"pT": pT_full, "qT": np.ascontiguousarray(fq[c * QS : (c + 1) * QS].T)}
        for c in range(NCORES)
    ]


def _host_top1(resA_results, p1, p2, fq):
    """Screen device maxima, rescore candidates in fp64, return jglob[1024]."""
    gm = np.stack([np.asarray(resA_results[c]["gmax"]) for c in range(NCORES)])
    ab = np.stack(
        [np.asarray(resA_results[c]["accb"]).astype(np.float32) for c in range(NCORES)]
    )
    # [c, 128p, t, ...] -> row r = t*128 + p
    gm = gm.transpose(0, 2, 1, 3, 4).reshape(NCORES, B2, 3, 2048 // GROUP)
    ab = ab.transpose(0, 2, 1, 3, 4).reshape(NCORES, B2, 3, 2048)
    if FP8:
        gm /= np.float32(Q8_SCALE)
        ab /= np.float32(Q8_SCALE)

    delta = DELTA_8 if FP8 else DELTA_R
    Mt = np.maximum(gm.max(axis=(0, 2, 3)), ab.max(axis=(0, 2, 3)))  # [B2]
    eps = 2 * delta + np.abs(Mt) * 2.0 ** -8 + 1e-4
    thresh = Mt - eps

    dir_b = np.asarray(DIRECT_BS, dtype=np.int64)  # [NT, 3]
    act_b = np.asarray(ACT_BS, dtype=np.int64)  # [NT, 3]
    rows_list, pos_list = [], []
    c_i, r_i, s_i, g_i = np.nonzero(gm >= thresh[None, :, None, None])
    if len(c_i):
        base = (
            c_i.astype(np.int64) * QS
            + dir_b[r_i // 128, s_i] * 2048
            + g_i.astype(np.int64) * GROUP
        )
        pos = (base[:, None] + np.arange(GROUP, dtype=np.int64)[None, :]).reshape(-1)
        rows = np.repeat(r_i.astype(np.int64), GROUP)
        rows_list.append(rows)
        pos_list.append(pos)
    c_i, r_i, s_i, p_i = np.nonzero(ab >= thresh[None, :, None, None])
    if len(c_i):
        pos = (
            c_i.astype(np.int64) * QS
            + act_b[r_i // 128, s_i] * 2048
            + p_i.astype(np.int64)
        )
        rows = r_i.astype(np.int64)
        rows_list.append(rows)
        pos_list.append(pos)
    rows = np.concatenate(rows_list)
    pos = np.concatenate(pos_list)

    P64 = np.concatenate([p1, p2], axis=0).astype(np.float64)
    s = np.einsum("kd,kd->k", fq[pos].astype(np.float64), P64[rows])

    # first-occurrence argmax per row: sort by (row, pos), take first pos
    # attaining the row max
    order = np.lexsort((pos, rows))
    rows_s, pos_s, s_s = rows[order], pos[order], s[order]
    jglob = np.empty(B2, dtype=np.int64)
    starts = np.searchsorted(rows_s, np.arange(B2), side="left")
    ends = np.searchsorted(rows_s, np.arange(B2), side="right")
    for r in range(B2):
        sl = slice(starts[r], ends[r])
        sv = s_s[sl]
        jglob[r] = pos_s[sl][np.argmax(sv)]
    return jglob


def _prep_C_inmaps(p1, p2, nn, temp):
    def l2n(x):
        n = np.sqrt((x.astype(np.float64) ** 2).sum(axis=1, keepdims=True))
        return (x / np.maximum(n, 1e-12)).astype(np.float32)

    p1n = l2n(p1)
    p2n = l2n(p2)
    inv_t = np.float32(1.0) / np.float32(temp)
    p1s = (p1n * inv_t).astype(np.float32)
    p2s = (p2n * inv_t).astype(np.float32)
    nn1, nn2 = nn[:B], nn[B:]
    nn1_adj = ((nn1 - p1n) + p1n).astype(np.float32)
    nn2_adj = ((nn2 - p2n) + p2n).astype(np.float32)

    mats = [(nn1_adj, p2s), (p2s, nn1_adj), (nn2_adj, p1s), (p1s, nn2_adj)]
    in_maps = []
    for c in range(NCORES):
        m = c // 2
        i0 = (c % 2) * 2
        lhs, rhs = mats[m]
        lhsT = np.ascontiguousarray(lhs.T[:, i0 * 128 : (i0 + 2) * 128])
        rhsT = np.ascontiguousarray(rhs.T)
        lhsN = np.ascontiguousarray(
            lhs.reshape(4, 128, D)[i0 : i0 + 2].transpose(1, 0, 2)
        )
        rhsN = np.ascontiguousarray(
            rhs.reshape(4, 128, D)[i0 : i0 + 2].transpose(1, 0, 2)
        )
        in_maps.append({"lhsT": lhsT, "rhsT": rhsT, "lhsN": lhsN, "rhsN": rhsN})
    return in_maps


def kernel(projections_1, projections_2, feature_queue, temperature):
    from concourse.bass_utils import run_bass_kernel_spmd

    p1 = np.ascontiguousarray(projections_1, dtype=np.float32)
    p2 = np.ascontiguousarray(projections_2, dtype=np.float32)
    fq = np.ascontiguousarray(feature_queue, dtype=np.float32)

    ncA = _get_nc("A")
    resA = run_bass_kernel_spmd(ncA, _prep_A_inmaps(p1, p2, fq), core_ids=list(range(NCORES)))
    jglob = _host_top1(resA.results, p1, p2, fq)
    nn = fq[jglob]

    ncC = _get_nc("C")
    resC = run_bass_kernel_spmd(
        ncC, _prep_C_inmaps(p1, p2, nn, temperature), core_ids=list(range(NCORES))
    )
    loss = np.empty(4 * B, dtype=np.float32)
    for c in range(NCORES):
        out = np.asarray(resC.results[c]["loss"], dtype=np.float32)  # [128, 2]
        for j in range(2):
            rt = 2 * c + j
            loss[rt * 128 : (rt + 1) * 128] = out[:, j]
    return loss
